# revision 24
# baseline (speedup 1.0000x reference)
"""MaxK-GCN (2-layer GraphConv) Bass kernel for 8 Trainium2 NeuronCores.

Strategy (graph/data parallel, per the sharding hint):
  - Nodes are partitioned across the 8 cores by contiguous range (12500 real
    rows/core, padded to 12544 = 98*128 table rows).
  - Dense phases (x@W_in, h@W, MaxK) run on each core for its own node rows in
    feature-major layout; all matmuls are fp16 (PSUM accumulates fp32).
  - The per-layer activation table z (fp16) is replicated via AllGather, then
    each core gathers the source rows for its dst-partitioned edges with
    dma_gather (4 SWDGE queues in parallel, trailing pad indices trimmed via
    -1 sentinels) and segment-sums them into per-256-node-window PSUM tiles on
    the PE using host-precomputed fp8 one-hot selection matrices streamed from
    DRAM.
  - Degree normalization is exact: deg_out^-1/2 and the window's deg_in^-1/2
    fold into per-partition activation scales; biases ride matmuls against
    deg_in^{+1/2} rows so the scale distributes correctly.

Self-contained: hardcodes the problem shapes; only needs numpy + the
concourse (Bass) stack that is installed in the environment.
"""

from contextlib import ExitStack

import numpy as np

# ---------------------------------------------------------------------------
# problem constants (nn_GCN_11768210391434)
# ---------------------------------------------------------------------------
N_NODES = 100000
D_IN = 128
D_HID = 128
D_OUT = 64
TOPK = 32
N_CORES = 8
P = 128
R_RANGES = 4  # gather ranges; rows-per-range must stay < 32768 (int16 idx)
NEG_SENTINEL = -1.0e30
NEG_TEST = -1.0e29
WDST = 256  # dst-window width (nodes)
N_QUEUES = 4  # SWDGE queues for parallel gather descriptor generation
MSG_BUFS = 14


def _cdiv(a, b):
    return (a + b - 1) // b


def _roundup(a, b):
    return _cdiv(a, b) * b


# ---------------------------------------------------------------------------
# host-side preprocessing: shard nodes, sort/pad edges, build device inputs
# ---------------------------------------------------------------------------
class Plan:
    pass


def make_plan(inputs, n_cores=N_CORES):
    import ml_dtypes
    import concourse.mybir as mybir

    f16 = np.float16
    f8 = mybir.dt.np(mybir.dt.float8e4)

    x = np.ascontiguousarray(np.asarray(inputs["x"], dtype=np.float32))
    src = np.asarray(inputs["src"]).astype(np.int64).ravel()
    dst = np.asarray(inputs["dst"]).astype(np.int64).ravel()
    N = x.shape[0]
    C = n_cores

    p = Plan()
    p.N, p.C = N, C
    p.din = x.shape[1]
    p.dhid = np.asarray(inputs["W1"]).shape[0]
    p.dout = np.asarray(inputs["W_out"]).shape[1]
    p.npc = _cdiv(N, C)                     # real nodes per core
    p.tpc = _cdiv(p.npc, P)                 # node tiles per core
    p.wdst = WDST
    p.tpc = _roundup(p.tpc, p.wdst // P)    # tiles group into windows
    p.npcp = p.tpc * P                      # padded rows per core
    p.nwin = p.npcp // p.wdst               # dst windows per core
    p.ntot = C * p.npcp                     # table rows
    p.R = R_RANGES
    p.nag = 16                              # AllGather chunks (4 per range):
    # finer chunks shrink the dense1->first-gather head latency (the last
    # chunk of range 0 gates the first gathers)
    assert p.npcp % p.nag == 0
    p.agrows = p.npcp // p.nag              # rows per core per AG chunk
    p.rs = p.C * p.agrows * (p.nag // p.R)  # table rows per gather range
    assert p.rs <= 32767

    # degrees over the full graph
    deg_out = np.maximum(np.bincount(src, minlength=N), 1).astype(np.float32)
    deg_in = np.maximum(np.bincount(dst, minlength=N), 1).astype(np.float32)
    dosc = deg_out ** -0.5
    disc = deg_in ** -0.5

    # node -> table row (range-interleaved: AG chunk q holds rows
    # [q*C*qrows, (q+1)*C*qrows) = all cores' local rows [q*qrows,(q+1)*qrows))
    core_of = np.minimum(src // p.npc, C - 1)
    lsrc = src - core_of * p.npc
    q_e = lsrc // p.agrows                  # AG chunk of each edge's src
    srow = (q_e * C + core_of) * p.agrows + (lsrc - q_e * p.agrows)
    r_e = q_e // (p.nag // p.R)             # gather range of each edge

    ecore = np.minimum(dst // p.npc, C - 1)
    ldst = dst - ecore * p.npc
    win = ldst // p.wdst
    drel = ldst - win * p.wdst

    GK = p.nwin * p.R                       # groups per core
    gkey = win * p.R + r_e
    gid = ecore * GK + gkey                 # global group id
    # group-major order, src-row ascending within each group: the gather's
    # HBM reads then stream in ascending address order (row locality).
    order = np.lexsort((srow, gid))
    gid_s = gid[order]

    counts = np.bincount(gid_s, minlength=C * GK).reshape(C, GK)
    gsz = np.maximum(P, _roundup(counts.max(axis=0), P))  # per-group slots
    p.gsz = gsz.astype(np.int64)            # static per-group-index sizes
    p.goff = np.concatenate([[0], np.cumsum(p.gsz)])  # slot offsets
    p.epad = int(p.goff[-1])                # edge slots per core per layer
    p.nch = p.epad // P                     # chunks per core per layer
    p.maxnch = int(p.gsz.max()) // P        # chunks in the largest group

    starts = np.concatenate([[0], np.cumsum(counts.ravel())])
    offs = np.arange(len(order), dtype=np.int64) - starts[gid_s]
    slot = p.goff[gid_s % GK] + offs        # slot within the core's edge array

    # pad slots gather row 0 (harmless; msel row is 0 there). Trailing -1
    # trimming is NOT used: it wedges the Q7 gather at full scale (confirmed
    # again 2026-08-11, with single_packet=False).
    idx16 = np.zeros((C, p.epad), dtype=np.int16)
    drel_a = np.full((C, p.epad), -1, dtype=np.int64)
    ec_s = gid_s // GK
    idx16[ec_s, slot] = (srow[order] - r_e[order] * p.rs).astype(np.int16)
    drel_a[ec_s, slot] = drel[order]

    p.dstrel = []
    # per-window chunk offsets (for the msel slab DMA)
    p.woff = [int(p.goff[w * p.R] // P) for w in range(p.nwin)] + [p.nch]

    # per-core packed layouts
    p.xT_core = []
    p.dosc_core = []      # deg_out^-1/2 per node, [P, tpc] (layer-1 z scale)
    p.dd_core = []        # dosc*disc per node, [P, tpc] (layer-2 z scale)
    p.disc_core = []      # disc per node, [P, tpc] (output scale)
    p.invd_core = []      # deg_in^{+1/2} per node, [1, npcp] fp16
    p.idx_core = []
    p.msel_core = []
    for c in range(C):
        xc = np.zeros((p.npcp, p.din), dtype=np.float32)
        lo, hi = c * p.npc, min((c + 1) * p.npc, N)
        xc[: hi - lo] = x[lo:hi]
        p.xT_core.append(np.ascontiguousarray(xc.T).astype(f16))

        do = np.ones(p.npcp, dtype=np.float32)
        do[: hi - lo] = dosc[lo:hi]
        di = np.ones(p.npcp, dtype=np.float32)
        di[: hi - lo] = disc[lo:hi]
        p.dosc_core.append(np.ascontiguousarray(do.reshape(p.tpc, P).T))
        p.dd_core.append(np.ascontiguousarray((do * di).reshape(p.tpc, P).T))
        p.disc_core.append(np.ascontiguousarray(di.reshape(p.tpc, P).T))
        p.invd_core.append((1.0 / di).astype(f16).reshape(1, p.npcp))

        iw = idx16[c].reshape(p.epad // 16, 16).T        # [16, epad/16]
        p.idx_core.append(np.ascontiguousarray(np.tile(iw, (P // 16, 1))))

        # fp8 one-hot selection matrices: [P, nch, wdst]
        ms = np.zeros((p.nch, P, p.wdst), dtype=np.uint8)
        dr = drel_a[c].reshape(p.nch, P)
        ch_i, sl_i = np.nonzero(dr >= 0)
        one = np.ones((), dtype=f8).view(np.uint8)
        ms[ch_i, sl_i, dr[ch_i, sl_i]] = one
        p.msel_core.append(
            np.ascontiguousarray(ms.transpose(1, 0, 2)).view(f8)
        )
        p.dstrel.append(np.ascontiguousarray(
            dr.astype(np.float32).reshape(p.nch, P).T))

    # shared (replicated) tensors
    W_in = np.asarray(inputs["W_in"], dtype=np.float64)
    W1 = np.asarray(inputs["W1"], dtype=np.float64)
    W2 = np.asarray(inputs["W2"], dtype=np.float64)
    W_out = np.asarray(inputs["W_out"], dtype=np.float64)
    b1 = np.asarray(inputs["b1"], dtype=np.float64)
    b2 = np.asarray(inputs["b2"], dtype=np.float64)
    bg1 = np.asarray(inputs["bg1"], dtype=np.float64)
    bg2 = np.asarray(inputs["bg2"], dtype=np.float64)
    b_out = np.asarray(inputs["b_out"], dtype=np.float64)

    p.W_in = W_in.astype(f16)
    p.W1 = W1.astype(f16)
    p.W2 = W2.astype(f16)
    p.W_out = W_out.astype(f16)
    p.b_in = np.asarray(inputs["b_in"], dtype=np.float32).reshape(p.dhid, 1)
    p.iota = np.tile(np.arange(p.wdst, dtype=np.float16).reshape(1, p.wdst), (P, 1))
    p.b1b = b1.astype(f16).reshape(1, p.dhid)
    # layer-2 dense bias folds the layer-1 graph-conv bias: b2' = b2 + bg1@W2
    p.b2b = (b2 + bg1 @ W2).astype(f16).reshape(1, p.dhid)
    # output bias folds the layer-2 graph-conv bias: b' = b_out + bg2@W_out
    p.bob = (b_out + bg2 @ W_out).astype(f16).reshape(1, p.dout)
    return p


def make_in_maps(p):
    maps = []
    for c in range(p.C):
        maps.append(
            {
                "xT": p.xT_core[c],
                "dosc": p.dosc_core[c],
                "dd": p.dd_core[c],
                "disc": p.disc_core[c],
                "invd": p.invd_core[c],
                "idx": p.idx_core[c],
                "msel": p.msel_core[c],
                "dstrel": p.dstrel[c],
                "iota": p.iota,
                "w_in": p.W_in,
                "w1": p.W1,
                "w2": p.W2,
                "w_out": p.W_out,
                "b_in": p.b_in,
                "b1b": p.b1b,
                "b2b": p.b2b,
                "bob": p.bob,
            }
        )
    return maps


def assemble_output(p, results):
    out = np.empty((p.N, p.dout), dtype=np.float32)
    for c in range(p.C):
        lo, hi = c * p.npc, min((c + 1) * p.npc, p.N)
        out[lo:hi] = results[c]["out"][: hi - lo]
    return out


# ---------------------------------------------------------------------------
# device program
# ---------------------------------------------------------------------------
def build_program(p, phases=("b1", "ag1", "c1", "ag2", "c2")):
    import concourse.mybir as mybir
    import concourse.tile as tile
    from concourse import bacc

    F32 = mybir.dt.float32
    F16 = mybir.dt.float16
    BF16 = mybir.dt.bfloat16
    F8 = mybir.dt.float8e4
    AF = mybir.ActivationFunctionType
    ALU = mybir.AluOpType

    nc = bacc.Bacc(
        "TRN2", target_bir_lowering=False, debug=False, num_devices=p.C,
        num_swdge_queues=N_QUEUES,
    )

    def din(name, shape, dt=F32):
        return nc.dram_tensor(name, shape, dt, kind="ExternalInput").ap()

    xT_d = din("xT", [p.din, p.npcp], F16)
    dosc_d = din("dosc", [P, p.tpc])
    dd_d = din("dd", [P, p.tpc])
    disc_d = din("disc", [P, p.tpc])
    invd_d = din("invd", [1, p.npcp], F16)
    idx_d = din("idx", [P, p.epad // 16], mybir.dt.int16)
    msel_d = din("msel", [P, p.nch, p.wdst], F8)
    dstrel_d = din("dstrel", [P, p.nch])
    iota_d = din("iota", [P, p.wdst], F16)
    w_in_d = din("w_in", [p.din, p.dhid], F16)
    w1_d = din("w1", [p.dhid, p.dhid], F16)
    w2_d = din("w2", [p.dhid, p.dhid], F16)
    w_out_d = din("w_out", [p.dhid, p.dout], F16)
    b_in_d = din("b_in", [p.dhid, 1])
    b1b_d = din("b1b", [1, p.dhid], F16)
    b2b_d = din("b2b", [1, p.dhid], F16)
    bob_d = din("bob", [1, p.dout], F16)

    out_d = nc.dram_tensor("out", [p.npcp, p.dout], F32, kind="ExternalOutput").ap()

    zloc = [nc.dram_tensor(f"z{i}loc", [p.npcp, p.dhid], BF16).ap() for i in (1, 2)]
    ztab = [
        nc.dram_tensor(f"Z{i}", [p.ntot, p.dhid], BF16, addr_space="Shared").ap()
        for i in (1, 2)
    ]
    rgroups = [list(range(p.C))]

    with tile.TileContext(nc) as tc, ExitStack() as ctx:
        cpool = ctx.enter_context(tc.tile_pool(name="const", bufs=1))

        _cn = [0]

        def const(ap_d, shape, dt=F32):
            _cn[0] += 1
            t = cpool.tile(shape, dt, tag=f"const{_cn[0]}")
            nc.sync.dma_start(t[:], ap_d)
            return t

        xT_sb = const(xT_d, [p.din, p.npcp], F16)
        w_in_sb = const(w_in_d, [p.din, p.dhid], F16)
        w1_sb = const(w1_d, [p.dhid, p.dhid], F16)
        w2_sb = const(w2_d, [p.dhid, p.dhid], F16)
        w_out_sb = const(w_out_d, [p.dhid, p.dout], F16)
        b_in_sb = const(b_in_d, [p.dhid, 1])
        b1row_sb = const(b1b_d, [1, p.dhid], F16)
        b2row_sb = const(b2b_d, [1, p.dhid], F16)
        bobrow_sb = const(bob_d, [1, p.dout], F16)
        ones_sb = cpool.tile([1, P], F16, tag="ones1")
        nc.vector.memset(ones_sb[:], 1.0)
        dosc_sb = const(dosc_d, [P, p.tpc])
        dd_sb = const(dd_d, [P, p.tpc])
        disc_sb = const(disc_d, [P, p.tpc])
        invd_sb = const(invd_d, [1, p.npcp], F16)
        idx_sb = cpool.tile([P, p.epad // 16], mybir.dt.int16)
        nc.sync.dma_start(idx_sb[:], idx_d)
        dstrel_sb = const(dstrel_d, [P, p.nch])
        iota_sb = const(iota_d, [P, p.wdst], F16)

        # pools
        hp = ctx.enter_context(tc.tile_pool(name="h", bufs=4))
        zsbp = ctx.enter_context(tc.tile_pool(name="zsb", bufs=3))
        wkp = ctx.enter_context(tc.tile_pool(name="wk", bufs=4))
        m8p = ctx.enter_context(tc.tile_pool(name="m8", bufs=8))
        mkp = ctx.enter_context(tc.tile_pool(name="mask", bufs=3))
        znp = ctx.enter_context(tc.tile_pool(name="zn", bufs=3))
        msgp = ctx.enter_context(tc.tile_pool(name="msg", bufs=MSG_BUFS))
        mslp = ctx.enter_context(tc.tile_pool(name="msl", bufs=5))
        msbp = ctx.enter_context(tc.tile_pool(name="msb", bufs=8))
        osbp = ctx.enter_context(tc.tile_pool(name="osb", bufs=3))
        ps_a = ctx.enter_context(tc.tile_pool(name="psA", bufs=4, space="PSUM"))
        ps_g = ctx.enter_context(tc.tile_pool(name="psG", bufs=4, space="PSUM"))

        # zero the msg pool buffers once: trimmed (pad) gather slots leave
        # stale SBUF bytes; zeroing keeps them finite (msel rows are 0 there).
        for _ in range(MSG_BUFS):
            mz = msgp.tile([P, p.maxnch, p.dhid], BF16)
            nc.vector.memset(mz[:], 0.0)

        def maxk_and_store(z_sb, t, zloc_ap):
            """MaxK(z) -> fp16 -> zloc rows of tile t. z_sb is pre-scaled by a
            positive per-node factor (preserves per-row top-k order).

            """
            cur = z_sb
            for r in range(TOPK // 8):
                m8 = m8p.tile([P, 8], F32)
                nc.vector.max(m8[:], cur[:])
                nxt = wkp.tile([P, P], BF16, tag=f"wk{r % 2}")
                nc.vector.match_replace(nxt[:], m8[:], cur[:], NEG_SENTINEL)
                cur = nxt
            mask = mkp.tile([P, P], BF16)
            nc.vector.tensor_scalar(
                mask[:], cur[:], NEG_TEST, None, op0=ALU.is_le,
            )
            znt = znp.tile([P, P], BF16, tag="znt")
            nc.vector.tensor_tensor(znt[:], z_sb[:], mask[:], op=ALU.mult)
            nc.sync.dma_start(zloc_ap[t * P : (t + 1) * P, :], znt[:])

        def dense_layer_tile(hT, t, w_sb, brow_sb, sc_sb, zloc_ap, use_invd):
            """z = maxk((h @ W + brow) * sc) for one 128-node tile; h in
            feature-major layout [feat, nodes] (fp16). When use_invd, the bias
            row rides an invd x brow matmul so a later *disc scale in sc
            distributes correctly."""
            z_ps = ps_g.tile([P, p.dhid], F32, tag="g")
            hT_ap = hT[:] if hasattr(hT, "tile") or hasattr(hT, "pool") else hT
            nc.tensor.matmul(z_ps[:], lhsT=hT_ap, rhs=w_sb[:], start=True, stop=False)
            brow_lhs = (
                invd_sb[:, t * P : (t + 1) * P] if use_invd else ones_sb[:]
            )
            nc.tensor.matmul(
                z_ps[:], lhsT=brow_lhs, rhs=brow_sb[:], start=False, stop=True
            )
            z_sb = zsbp.tile([P, p.dhid], BF16)
            nc.scalar.activation(
                z_sb[:], z_ps[:], AF.Identity, scale=sc_sb[:, t : t + 1]
            )
            maxk_and_store(z_sb, t, zloc_ap)

        import os
        agg_mode = os.environ.get("GCN_AGG_MODE", "full")
        one_q = bool(os.environ.get("GCN_1Q"))
        _gq = [0]  # round-robin sub-gather queue counter

        def agg_window(w, ztab_ap, dve_msel=False):
            """Aggregate all edges of dst-window w from table -> h tile
            [feat, wdst] (feature-major, fp16). No bias or deg_in scale here;
            both fold into the consuming stage."""
            c0, c1 = p.woff[w], p.woff[w + 1]
            do_mm = agg_mode in ("full", "nogather")
            do_gather = agg_mode in ("full", "nomm")
            if do_mm:
                agg_ps = ps_a.tile([P, p.wdst], F32)
                msl = None
                if not dve_msel:
                    msl = mslp.tile([P, c1 - c0, p.wdst], F8)
                    eng = nc.sync if (w // 2) % 2 == 0 else nc.scalar
                    eng.dma_start(msl[:], msel_d[:, c0:c1, :])
            n_mm = c1 - c0
            mm = 0
            for r in range(p.R):
                g = w * p.R + r
                G_g = int(p.gsz[g])
                nch_g = G_g // P
                off = int(p.goff[g])
                lo = r * p.rs
                hi = min((r + 1) * p.rs, p.ntot)
                msg = msgp.tile([P, p.maxnch, p.dhid], BF16)
                if do_gather:
                    nc.gpsimd.dma_gather(
                        msg[:, :nch_g, :],
                        ztab_ap[lo:hi, :],
                        idx_sb[:, off // 16 : (off + G_g) // 16],
                        G_g,
                        G_g,
                        p.dhid,
                        single_packet=False,
                        queue_num=0 if one_q else r % N_QUEUES,
                    )
                if do_mm:
                    for k in range(nch_g):
                        ci = off // P + k
                        if dve_msel:
                            mst = msbp.tile([P, p.wdst], F16)
                            nc.vector.tensor_scalar(
                                mst[:], iota_sb[:],
                                dstrel_sb[:, ci : ci + 1], None,
                                op0=ALU.is_equal,
                            )
                            rhs_ap = mst[:]
                        else:
                            rhs_ap = msl[:, ci - c0, :]
                        nc.tensor.matmul(
                            agg_ps[:], lhsT=msg[:, k, :], rhs=rhs_ap,
                            start=(mm == 0), stop=(mm == n_mm - 1),
                        )
                        mm += 1
            h_sb = hp.tile([P, p.wdst], F16)
            if do_mm:
                nc.scalar.activation(h_sb[:], agg_ps[:], AF.Identity)
            else:
                nc.vector.memset(h_sb[:], 0.0)
            return h_sb

        # ---- phase B1: h1 = relu(W_in.T @ xT + b), z1 = maxk(h1@W1+b1)*dosc
        with nc.named_scope("dense1"):
            for t in range(p.tpc):
                h1_ps = ps_g.tile([P, P], F32, tag="g")
                nc.tensor.matmul(
                    h1_ps[:], lhsT=w_in_sb[:],
                    rhs=xT_sb[:, t * P : (t + 1) * P],
                    start=True, stop=True,
                )
                h1 = hp.tile([P, P], F16, tag="h1")
                nc.scalar.activation(h1[:], h1_ps[:], AF.Relu, bias=b_in_sb[:, :1])
                dense_layer_tile(h1, t, w1_sb, b1row_sb, dosc_sb, zloc[0], False)

        with nc.named_scope("ag1"):
            for q in range(p.nag):
                nc.gpsimd.collective_compute(
                    "AllGather", mybir.AluOpType.bypass, replica_groups=rgroups,
                    ins=[zloc[0][q * p.agrows : (q + 1) * p.agrows, :]],
                    outs=[ztab[0][q * p.C * p.agrows : (q + 1) * p.C * p.agrows, :]],
                )

        # ---- phase C1: aggregate layer1, z2 = maxk((h2@W2+b2')*dosc*disc)
        if "c1" in phases:
          with nc.named_scope("agg1_dense2"):
            for w in range(p.nwin):
                h2 = agg_window(w, ztab[0])
                for j in range(p.wdst // P):
                    t = w * (p.wdst // P) + j
                    dense_layer_tile(
                        h2[:, j * P : (j + 1) * P], t, w2_sb, b2row_sb,
                        dd_sb, zloc[1], True,
                    )

        if "ag2" in phases:
          with nc.named_scope("ag2"):
            for q in range(p.nag):
                nc.gpsimd.collective_compute(
                    "AllGather", mybir.AluOpType.bypass, replica_groups=rgroups,
                    ins=[zloc[1][q * p.agrows : (q + 1) * p.agrows, :]],
                    outs=[ztab[1][q * p.C * p.agrows : (q + 1) * p.C * p.agrows, :]],
                )

        # ---- phase C2: aggregate layer2, out = (h3@W_out+b')*disc per node
        if "c2" in phases:
          with nc.named_scope("agg2_out"):
            for w in range(p.nwin):
                h3 = agg_window(w, ztab[1])
                for j in range(p.wdst // P):
                    t = w * (p.wdst // P) + j
                    o_ps = ps_g.tile([P, p.dout], F32, tag="g")
                    nc.tensor.matmul(
                        o_ps[:], lhsT=h3[:, j * P : (j + 1) * P],
                        rhs=w_out_sb[:], start=True, stop=False,
                    )
                    nc.tensor.matmul(
                        o_ps[:], lhsT=invd_sb[:, t * P : (t + 1) * P],
                        rhs=bobrow_sb[:], start=False, stop=True,
                    )
                    o_sb = osbp.tile([P, p.dout], F32)
                    nc.scalar.activation(
                        o_sb[:], o_ps[:], AF.Identity,
                        scale=disc_sb[:, t : t + 1],
                    )
                    nc.sync.dma_start(out_d[t * P : (t + 1) * P, :], o_sb[:])

    nc.compile()
    return nc


# ---------------------------------------------------------------------------
# entry points
# ---------------------------------------------------------------------------
def _install_axon_ntff_hook():
    """Register the NTFF profile hook that concourse's axon path looks for
    (the agent image's antenv lacks axon_hooks; shim it in)."""
    import sys
    import types

    try:
        from antenv.axon_hooks import get_axon_ntff_profile_hook  # noqa: F401

        return  # already available
    except ImportError:
        pass
    import antenv

    mod = types.ModuleType("antenv.axon_hooks")
    _state = {"hook": None}
    mod.set_axon_ntff_profile_hook = lambda h: _state.__setitem__("hook", h)
    mod.get_axon_ntff_profile_hook = lambda: _state["hook"]
    sys.modules["antenv.axon_hooks"] = mod
    antenv.axon_hooks = mod
    from trn_agent_boot.trn_boot import _ntff_profile_via_ctypes

    mod.set_axon_ntff_profile_hook(
        _ntff_profile_via_ctypes("/opt/axon/libaxon_pjrt.so")
    )


def run_gcn(inputs, n_cores=N_CORES, trace=False, trace_cores=None, **_ignored):
    from concourse.bass_utils import run_bass_kernel_spmd

    if trace:
        _install_axon_ntff_hook()
    import os
    p = make_plan(inputs, n_cores=n_cores)
    ph = os.environ.get("GCN_PHASES")
    nc = build_program(p, phases=tuple(ph.split(",")) if ph else ("b1", "ag1", "c1", "ag2", "c2"))
    in_maps = make_in_maps(p)
    bkr = run_bass_kernel_spmd(
        nc, in_maps, list(range(p.C)), trace=trace, trace_cores=trace_cores
    )
    out = assemble_output(p, bkr.results)
    return out, bkr, p, nc


def kernel(**inputs):
    out, _, _, _ = run_gcn(inputs)
    return out

